# revision 30
# baseline (speedup 1.0000x reference)
"""GAT/GCN message-passing layer on 8 Trainium2 NeuronCores.

Math: the reference computes, per query node i,
    e[i,j]   = f_src[i] + f_dst[j]           (GAT additive attention, masked by Ahat>0)
    attn     = softmax_j(e masked)
    out      = relu(attn @ h_prime)
The f_src[i] term is constant along the softmax axis, so it cancels:
    attn[i,j] = Ahat[i,j]*exp(f_dst[j]) / sum_k Ahat[i,k]*exp(f_dst[k])
With g = exp(f_dst) the whole layer is one GEMM over the adjacency:
    out = relu( (Ahat @ [g*h_prime | g])[:, :256] / (Ahat @ [g*h_prime | g])[:, 256:] )
where h_prime = node_feats @ w and f_dst = node_feats @ (w @ w_a @ a[2:4]).

Sharding: 1D row partition of query nodes.  Each of the 8 cores owns 1024 rows
of Ahat (fed pre-transposed, [8192, 1024], so the contraction axis j lands on
SBUF partitions) and computes its 1024x256 slice of the output.  The small
tensors (node_feats^T, [w | u]) are replicated; each core recomputes the
B = [g*h_prime | g] panel locally, which is cheaper than a collective.
"""

import os
import sys

import numpy as np

sys.path.insert(0, "/opt/trn_rl_repo")

import concourse.bass as bass  # noqa: E402
import concourse.tile as tile  # noqa: E402
from concourse.tile import add_dep_helper  # noqa: E402
from concourse import mybir  # noqa: E402
from concourse.bass_utils import run_bass_kernel_spmd  # noqa: E402

N = 8192
F = 256  # in_features == out_features
FE = F + 1  # h_prime columns + the g column
NCORES = 8
ROWS = N // NCORES  # 1024 output rows per core
P = 128
NJ = N // P  # 64 contraction blocks
NI = ROWS // P  # 8 output-row blocks per core

# Adjacency is binary, so bf16 is lossless for A; B is rounded to bf16
# (measured end-to-end rel-err ~1.6e-3 vs the fp32 reference).
A_DT = mybir.dt.bfloat16
A_NP = "bfloat16"
B_DT = mybir.dt.bfloat16

_CACHE = {}


def _build():
    nc = bass.Bass(
        "TRN2",
        target_bir_lowering=False,
        debug=False,
        enable_asserts=True,
        num_devices=NCORES,
    )
    aT = nc.dram_tensor("aT", [N, ROWS], A_DT, kind="ExternalInput").ap()
    # params = [node_feats^T | w | u] fused so a single DMA (one semaphore
    # lane) feeds the whole prefix: walrus caps sync waits per instruction.
    params = nc.dram_tensor(
        "params", [F, N + FE], mybir.dt.float32, kind="ExternalInput"
    ).ap()
    out = nc.dram_tensor("out", [ROWS, F], mybir.dt.float32, kind="ExternalOutput").ap()

    with tile.TileContext(nc) as tc:
        _body(tc, aT, params, out)
    return nc


def _body(tc, aT, params, out):
    nc = tc.nc
    f32 = mybir.dt.float32

    with (
        tc.tile_pool(name="consts", bufs=1) as consts,
        tc.tile_pool(name="bpool", bufs=NJ) as bpool,
        # apool bufs == SWDGE lane count: slot k is always rewritten by a DMA
        # on the same lane, so the WAW needs no sync wait (same-proc FIFO) and
        # each aT DMA carries only the single PE WAR wait walrus allows.
        tc.tile_pool(name="apool", bufs=1) as apool,
        tc.tile_pool(name="gpool", bufs=4) as gpool,
        tc.tile_pool(name="opool", bufs=8) as opool,
        tc.tile_pool(name="rpool", bufs=8) as rpool,
        tc.tile_pool(name="psum", bufs=1, space="PSUM") as psum,
    ):
        CJ = 2  # j-blocks per aT DMA -> 512 KB transfers near line rate
        NCHUNK = NJ // CJ
        NBUF = 8

        # Replicated params resident in SBUF, loaded by one DMA.
        params_sb = consts.tile([P, 2, N + FE], f32, tag="params")
        nc.sync.dma_start(params_sb[:], params.rearrange("(o p) f -> p o f", p=P))
        nfT_sb = [params_sb[:, kb, 0:N] for kb in range(2)]
        wext_sb = params_sb[:, :, N : N + FE]

        # Prefetch the first NBUF adjacency chunks before the prefix work
        # queues up on ACT (aT DMAs are ACT-issued HWDGE, see below).
        at_hist = []
        for jc in range(NBUF):
            at = apool.tile([P, CJ * ROWS], A_DT, tag=f"aT{jc % NBUF}", name=f"at{jc}")
            at_hist.append(at)
            nc.sync.dma_start(
                at.rearrange("p (o f) -> p o f", f=ROWS),
                aT[jc * CJ * P : (jc + 1) * CJ * P, :].rearrange(
                    "(o p) f -> p o f", p=P
                ),
            )

        # One PSUM accumulator bank per 128-row output block; the h_prime
        # prefix borrows banks 0/1 before the accumulation groups start.
        acc = [
            psum.tile([P, FE], f32, tag=f"acc{i}", name=f"acc{i}") for i in range(NI)
        ]

        # Prefix: B[j] = [g*h_prime | g] for every contraction block j.
        btiles = []
        for j in range(NJ):
            hp = acc[j % 2]
            for kb in range(2):
                nc.tensor.matmul(
                    hp[:],
                    lhsT=nfT_sb[kb][:, j * P : (j + 1) * P],
                    rhs=wext_sb[:, kb, :],
                    start=(kb == 0),
                    stop=(kb == 1),
                )
            # All B-prep on the scalar engine: PSUM-bank WAR deps then target a
            # single engine (walrus rejects instructions with many sync waits).
            g = gpool.tile([P, 1], f32, tag="g")
            nc.scalar.activation(g[:], hp[:, F : F + 1], mybir.ActivationFunctionType.Exp)
            b = bpool.tile([P, FE], B_DT, tag="B")
            nc.scalar.mul(b[:, 0:F], hp[:, 0:F], g[:])
            nc.scalar.copy(b[:, F : F + 1], g[:])
            btiles.append(b)

        # Main stream: acc[i] += aT_block(j,i)^T @ B[j], then normalize+relu.
        # walrus allows only ONE sync wait per DMA.  A refill DMA naturally
        # needs two (PE WAR on the evicted tile + old writer's DMA lane), so
        # we first emit a 1-element ACT write ("touch") on the evicted tile:
        # the touch, a compute op, may carry both waits, and the ACT-issued
        # refill DMA right after it inherits ACT's observed clock -> <=1 wait.
        last_mm = None
        last_act = None
        refills = []
        for jc in range(NCHUNK):
            at = at_hist[jc]
            for jj in range(CJ):
                j = jc * CJ + jj
                for i in range(NI):
                    last_mm = nc.tensor.matmul(
                        acc[i][:],
                        lhsT=at[:, jj * ROWS + i * P : jj * ROWS + (i + 1) * P],
                        rhs=btiles[j][:],
                        start=(j == 0),
                        stop=(j == NJ - 1),
                    )
            nxt = jc + NBUF
            if nxt < NCHUNK:
                # chunk nxt reuses the slot of the chunk just consumed (jc).
                # This walrus accepts only ONE sync wait per instruction, but a
                # refill DMA naturally needs two (PE WAR on the evicted tile +
                # the old writer's DMA lane).  Collapse them with two tiny ACT
                # ops on a stepped view touching the tile's first+last columns
                # (whose reader/writer ticks equal the full tile's): a read
                # absorbs the old-DMA dep, a zero-write then absorbs the PE
                # readers; the refill waits only on the latter, and vector
                # clock merging covers the rest.
                head = at[:, 0:1]
                tail = at[:, CJ * ROWS - 1 : CJ * ROWS]
                scr = gpool.tile([P, 1], A_DT, tag="scr")
                scr_last = scr
                rd = nc.scalar.copy(scr[:], head)
                wz = nc.scalar.mul(tail, tail, 0.0)
                add_dep_helper(wz.ins, rd.ins, sync=False, reason="read before zero")
                at_new = apool.tile(
                    [P, CJ * ROWS], A_DT, tag=f"aT{nxt % NBUF}", name=f"at{nxt}"
                )
                at_hist.append(at_new)
                # ACT-issued: the sequencer's observed clock (advanced by
                # rd/wz) covers the PE and DMA-lane deps, leaving one wait.
                refill = nc.scalar.dma_start(
                    at_new.rearrange("p (o f) -> p o f", f=ROWS),
                    aT[nxt * CJ * P : (nxt + 1) * CJ * P, :].rearrange(
                        "(o p) f -> p o f", p=P
                    ),
                )
                add_dep_helper(refill.ins, wz.ins, sync=False, reason="absorb refill deps")
                last_act = wz
                refills.append(refill)

        # Epilogue entirely on DVE so each op carries at most one wait (PE).
        # First advance DVE's observed ACT clock past the last ACT op (a read
        # of the final eviction's scr tile) so recips don't inherit stale ACT
        # deps from the prefix's use of PSUM banks 0/1.
        wscr = gpool.tile([P, 1], A_DT, tag="wscr")
        warm = nc.vector.tensor_copy(wscr[:], scr_last[:])
        otile = opool.tile([P, NI * F], f32, tag="o")
        last_dve = None
        for i in range(NI):
            recip = rpool.tile([P, 1], f32, tag="recip", name=f"recip{i}")
            rc = nc.vector.reciprocal(recip[:], acc[i][:, F : F + 1])
            if i == 0:
                add_dep_helper(rc.ins, warm.ins, sync=False, reason="warm first")
            o = otile[:, i * F : (i + 1) * F]
            nc.vector.tensor_scalar_mul(o, acc[i][:, 0:F], recip[:])
            last_dve = nc.vector.tensor_scalar_max(o, o, 0.0)
        # One SWDGE store for the whole 1024x256 slice: a single DVE wait.
        store = nc.gpsimd.dma_start(
            out.rearrange("(i p) f -> p i f", p=P),
            otile.rearrange("p (i f) -> p i f", f=F),
        )

        # Funnel every proc's final tick into SP via single-wait nops, so the
        # kernel-tail drain (which otherwise aggregates ~19 sem waits, far
        # over walrus's per-instruction cap) has nothing left to wait on.
        for dep in [*refills[-NBUF:], store, last_mm, last_act, warm, last_dve]:
            nop = nc.sync.nop(nofuse=True, hint="tail_funnel")
            add_dep_helper(nop.ins, dep.ins, reason="tail funnel")


def _prep_inputs(node_feats, Ahat, w, w_a, a):
    node_feats = np.asarray(node_feats, dtype=np.float32)
    Ahat = np.asarray(Ahat, dtype=np.float32)
    w = np.asarray(w, dtype=np.float32)
    w_a = np.asarray(w_a, dtype=np.float32)
    a = np.asarray(a, dtype=np.float32)

    u = w @ (w_a @ a[2:4])  # [256, 1]
    wext = np.concatenate([w, u], axis=1)  # [256, 257]
    nfT = node_feats.T  # [256, 8192]
    params = np.ascontiguousarray(np.concatenate([nfT, wext], axis=1))  # [256, 8449]

    in_maps = []
    for c in range(NCORES):
        aT_c = np.ascontiguousarray(Ahat[c * ROWS : (c + 1) * ROWS, :].T.astype(A_NP))
        in_maps.append({"aT": aT_c, "params": params})
    return in_maps


def _run(inputs, trace=False, **kwargs):
    if "nc" not in _CACHE:
        _CACHE["nc"] = _build()
    nc = _CACHE["nc"]
    in_maps = _prep_inputs(**inputs)
    res = run_bass_kernel_spmd(
        nc, in_maps, core_ids=list(range(NCORES)), trace=trace, **kwargs
    )
    full = np.concatenate([res.results[c]["out"] for c in range(NCORES)], axis=0)
    return full, res


def kernel(**inputs) -> np.ndarray:
    out, _ = _run(inputs, trace=False)
    return out


# revision 36
# speedup vs baseline: 1.3133x; 1.3133x over previous
"""GAT/GCN message-passing layer on 8 Trainium2 NeuronCores.

Math: the reference computes, per query node i,
    e[i,j]   = f_src[i] + f_dst[j]           (GAT additive attention, masked by Ahat>0)
    attn     = softmax_j(e masked)
    out      = relu(attn @ h_prime)
The f_src[i] term is constant along the softmax axis, so it cancels:
    attn[i,j] = Ahat[i,j]*exp(f_dst[j]) / sum_k Ahat[i,k]*exp(f_dst[k])
With g = exp(f_dst) the whole layer is one GEMM over the adjacency:
    out = relu( (Ahat @ [g*h_prime | g])[:, :256] / (Ahat @ [g*h_prime | g])[:, 256:] )
where h_prime = node_feats @ w and f_dst = node_feats @ (w @ w_a @ a[2:4]).

Sharding: 1D row partition of query nodes.  Each of the 8 cores owns 1024 rows
of Ahat (fed pre-transposed, [8192, 1024], so the contraction axis j lands on
SBUF partitions) and computes its 1024x256 slice of the output.  The small
tensors (node_feats^T, [w | u]) are replicated; each core recomputes the
B = [g*h_prime | g] panel locally, which is cheaper than a collective.

The adjacency is binary so bf16 is lossless for it; everything else is bf16 in
/ fp32-PSUM accumulate (measured end-to-end rel-err ~2e-3 vs fp32 reference).

A quirk this kernel works around everywhere: this walrus accepts only ONE sync
wait per instruction, so the dataflow is arranged so every instruction has at
most one cross-engine dependency (single-engine op chains, ACT-issued DMAs
that inherit the scalar sequencer's observed clock, and read/write "absorber"
ops ahead of DMAs that would otherwise need two waits).
"""

import os
import sys

import numpy as np

sys.path.insert(0, "/opt/trn_rl_repo")

import concourse.bass as bass  # noqa: E402
import concourse.tile as tile  # noqa: E402
from concourse import mybir  # noqa: E402
from concourse.bass_utils import run_bass_kernel_spmd  # noqa: E402
from concourse.tile import add_dep_helper  # noqa: E402

N = 8192
F = 256  # in_features == out_features
FE = F + 1  # h_prime columns + the g column
NCORES = 8
ROWS = N // NCORES  # 1024 output rows per core
P = 128
NJ = N // P  # 64 contraction blocks
NI = ROWS // P  # 8 output-row blocks per core

BF = mybir.dt.bfloat16

# j-block layout of the adjacency stream:
#   pinned tiles: j-blocks 0..JSTART-1 stay resident (PJ j-blocks per tile)
#   rotating ring: j-blocks JSTART..63 through NBUF slots (CJ j-blocks per DMA)
PJ = 8
NPIN = 4
JSTART = NPIN * PJ  # 32
CJ = 2
NROT = (NJ - JSTART) // CJ  # 16
NBUF = 8

_CACHE = {}


def _build():
    nc = bass.Bass(
        "TRN2",
        target_bir_lowering=False,
        debug=False,
        enable_asserts=True,
        num_devices=NCORES,
    )
    aT = nc.dram_tensor("aT", [N, ROWS], BF, kind="ExternalInput").ap()
    # params = [w | u | node_feats^T], all bf16, loaded in column chunks
    params = nc.dram_tensor("params", [F, FE + N], BF, kind="ExternalInput").ap()
    out = nc.dram_tensor("out", [ROWS, F], mybir.dt.float32, kind="ExternalOutput").ap()

    with tile.TileContext(nc) as tc:
        _body(tc, aT, params, out)
    return nc


def _body(tc, aT, params, out):
    nc = tc.nc
    f32 = mybir.dt.float32
    Exp = mybir.ActivationFunctionType.Exp

    with (
        tc.tile_pool(name="consts", bufs=1) as consts,
        tc.tile_pool(name="apool", bufs=1) as apool,
        tc.tile_pool(name="gpool", bufs=4) as gpool,
        tc.tile_pool(name="opool", bufs=1) as opool,
        tc.tile_pool(name="rpool", bufs=8) as rpool,
        tc.tile_pool(name="psum", bufs=1, space="PSUM") as psum,
    ):
        # ---- loads ---------------------------------------------------------
        # params in NCHP column chunks, ACT-issued so the prefix matmuls'
        # dependencies (chunk DMA + PSUM-bank WAR + B reads) all collapse onto
        # the single Activation semaphore.
        params_sb = consts.tile([P, 2, FE + N], BF, tag="params")
        NCHP = 8
        csize = (FE + N) // NCHP  # 1056 + remainder on chunk 0
        bounds = [0] + [FE + N - (NCHP - 1 - c) * csize for c in range(NCHP)]
        for c in range(NCHP):
            lo, hi = bounds[c], bounds[c + 1]
            nc.scalar.dma_start(
                params_sb[:, :, lo:hi],
                params[:, lo:hi].rearrange("(o p) f -> p o f", p=P),
            )
        wext_sb = params_sb[:, :, 0:FE]
        nfT_sb = [params_sb[:, kb, FE : FE + N] for kb in range(2)]

        # Adjacency: 4 pinned 4MB loads (j-blocks 0..31) + rotating ring.
        pinned = []
        for t in range(NPIN):
            pt = consts.tile([P, PJ * ROWS], BF, tag=f"aTp{t}", name=f"aTp{t}")
            pinned.append(pt)
            nc.sync.dma_start(
                pt.rearrange("p (o f) -> p o f", f=ROWS),
                aT[t * PJ * P : (t + 1) * PJ * P, :].rearrange(
                    "(o p) f -> p o f", p=P
                ),
            )
        rot = []
        for rc in range(NBUF):
            rt = apool.tile([P, CJ * ROWS], BF, tag=f"aTr{rc % NBUF}", name=f"aTr{rc}")
            rot.append(rt)
            nc.sync.dma_start(
                rt.rearrange("p (o f) -> p o f", f=ROWS),
                aT[(JSTART + rc * CJ) * P : (JSTART + (rc + 1) * CJ) * P, :].rearrange(
                    "(o p) f -> p o f", p=P
                ),
            )

        def a_block(j):
            """SBUF [128, 128] lhsT view of adjacency j-block, i-block i."""
            if j < JSTART:
                t = pinned[j // PJ]
                o = j % PJ
            else:
                t = rot[(j - JSTART) // CJ]
                o = (j - JSTART) % CJ
            return t, o

        # ---- PSUM accumulators --------------------------------------------
        acc = [
            psum.tile([P, FE], f32, tag=f"acc{i}", name=f"acc{i}") for i in range(NI)
        ]

        # ---- prefix: B[j] = [g*h_prime | g], all 64 j-blocks ---------------
        # h' matmuls borrow PSUM banks 0/1; those banks' accumulation groups
        # therefore start at j=JSTART in the main stream (their earlier
        # j-terms are backfilled at the end from the pinned tiles) so the
        # bank WAR against the last B-prep doesn't stall the PE queue.
        B_all = consts.tile([P, NJ * FE], BF, tag="B")
        btile = [B_all[:, j * FE : (j + 1) * FE] for j in range(NJ)]
        G = consts.tile([P, NJ], f32, tag="G")
        prev_act = None
        for j in range(NJ):
            hp = acc[j % 2]
            for kb in range(2):
                nc.tensor.matmul(
                    hp[:],
                    lhsT=nfT_sb[kb][:, j * P : (j + 1) * P],
                    rhs=wext_sb[:, kb, :],
                    start=(kb == 0),
                    stop=(kb == 1),
                )
            b = btile[j]
            gj = G[:, j : j + 1]
            ex = nc.scalar.activation(gj, hp[:, F : F + 1], Exp)
            if prev_act is not None:
                # keep B-prep in emission order on ACT; a scheduler shuffle
                # makes some of these ops pick up a second (same-sem) wait
                add_dep_helper(ex.ins, prev_act.ins, sync=False, reason="act order")
            prev_act = nc.scalar.mul(b[:, 0:F], hp[:, 0:F], gj)
            if j % 8 == 7:
                # one strided cast-copy drops this 8-group's g column into B
                c0 = j - 7
                prev_act = nc.scalar.copy(
                    B_all[:, c0 * FE + F : (j + 1) * FE : FE], G[:, c0 : j + 1]
                )

        # ---- main stream ---------------------------------------------------
        last_mm = None
        last_act = None
        refills = []
        scr_last = None
        for j in range(NJ):
            t, o = a_block(j)
            ilist = range(NI) if j >= JSTART else range(2, NI)
            for i in ilist:
                first = j == 0 or (j == JSTART and i < 2)
                last_mm = nc.tensor.matmul(
                    acc[i][:],
                    lhsT=t[:, o * ROWS + i * P : o * ROWS + (i + 1) * P],
                    rhs=btile[j][:],
                    start=first,
                    stop=(j == NJ - 1 and i >= 2),
                )
            # ring refill bookkeeping (rotating region only)
            if j >= JSTART and (j - JSTART) % CJ == CJ - 1:
                rc = (j - JSTART) // CJ
                nxt = rc + NBUF
                if nxt < NROT:
                    at = rot[rc]
                    # single-wait absorbers: ACT read (old DMA dep) + ACT
                    # zero-write (PE readers dep); ACT-issued refill then
                    # needs only one wait.
                    head = at[:, 0:1]
                    tail = at[:, CJ * ROWS - 1 : CJ * ROWS]
                    scr = gpool.tile([P, 1], BF, tag="scr")
                    scr_last = scr
                    rd = nc.scalar.copy(scr[:], head)
                    wz = nc.scalar.mul(tail, tail, 0.0)
                    add_dep_helper(wz.ins, rd.ins, sync=False, reason="rd<wz")
                    at_new = apool.tile(
                        [P, CJ * ROWS], BF, tag=f"aTr{nxt % NBUF}", name=f"aTr{nxt}"
                    )
                    rot.append(at_new)
                    refill = nc.scalar.dma_start(
                        at_new.rearrange("p (o f) -> p o f", f=ROWS),
                        aT[
                            (JSTART + nxt * CJ) * P : (JSTART + (nxt + 1) * CJ) * P, :
                        ].rearrange("(o p) f -> p o f", p=P),
                    )
                    add_dep_helper(refill.ins, wz.ins, sync=False, reason="rf<wz")
                    last_act = wz
                    refills.append(refill)

        # backfill: banks 0/1 take their j<JSTART terms from the pinned tiles
        for j in range(JSTART):
            t, o = a_block(j)
            for i in range(2):
                last_mm = nc.tensor.matmul(
                    acc[i][:],
                    lhsT=t[:, o * ROWS + i * P : o * ROWS + (i + 1) * P],
                    rhs=btile[j][:],
                    start=False,
                    stop=(j == JSTART - 1),
                )
        # close the accumulation groups of banks 2..7 (their last matmul above
        # had stop=False; emit the stop on a zero-contribution... not needed:
        # stop is a simulator bookkeeping flag, but keep groups well-formed by
        # marking the true last j=NJ-1 matmuls; handled below via epilogue.

        # ---- epilogue: out[i] = relu(acc[i][:, :F] / acc[i][:, F]) ---------
        # all DVE so each op carries at most one (PE) wait; the warm read
        # advances DVE's observed ACT clock first.
        # denominators: ACT copies each bank's g-sum into one SBUF tile (one
        # PE wait each; avoids DVE-reads-PSUM bank deps), one DVE reciprocal.
        denom8 = rpool.tile([P, NI], f32, tag="denom8")
        for i in range(NI):
            nc.scalar.copy(denom8[:, i : i + 1], acc[i][:, F : F + 1])
        recip8 = rpool.tile([P, NI], f32, tag="recip8")
        nc.vector.reciprocal(recip8[:], denom8[:])
        # sacrificial same-proc read: soaks up the redundant DVE wait Tile
        # pins on the first consumer of recip8
        rscr = rpool.tile([P, NI], f32, tag="rscr")
        nc.vector.tensor_copy(rscr[:], recip8[:])
        otile = opool.tile([P, NI * F], f32, tag="o")
        last_dve = None
        for i in range(NI):
            o = otile[:, i * F : (i + 1) * F]
            nc.vector.tensor_scalar_mul(o, acc[i][:, 0:F], recip8[:, i : i + 1])
            last_dve = nc.vector.tensor_scalar_max(o, o, 0.0)
        store = nc.gpsimd.dma_start(
            out.rearrange("(i p) f -> p i f", p=P),
            otile.rearrange("p (i f) -> p i f", f=F),
        )

        # Funnel every proc's final tick into SP via single-wait nops so the
        # kernel-tail drain (which otherwise aggregates ~19 sem waits, far
        # over walrus's cap) has nothing left to wait on.
        for dep in [*refills[-NBUF:], store, last_mm, last_act, last_dve]:
            nop = nc.sync.nop(nofuse=True, hint="tail_funnel")
            add_dep_helper(nop.ins, dep.ins, reason="tail funnel")


def _prep_inputs(node_feats, Ahat, w, w_a, a):
    node_feats = np.asarray(node_feats, dtype=np.float32)
    Ahat = np.asarray(Ahat, dtype=np.float32)
    w = np.asarray(w, dtype=np.float32)
    w_a = np.asarray(w_a, dtype=np.float32)
    a = np.asarray(a, dtype=np.float32)

    u = w @ (w_a @ a[2:4])  # [256, 1]
    params = np.concatenate([w, u, node_feats.T], axis=1).astype("bfloat16")
    params = np.ascontiguousarray(params)  # [256, 257 + 8192]

    in_maps = []
    for c in range(NCORES):
        aT_c = np.ascontiguousarray(
            Ahat[c * ROWS : (c + 1) * ROWS, :].T.astype("bfloat16")
        )
        in_maps.append({"aT": aT_c, "params": params})
    return in_maps


def _run(inputs, trace=False, **kwargs):
    if "nc" not in _CACHE:
        _CACHE["nc"] = _build()
    nc = _CACHE["nc"]
    in_maps = _prep_inputs(**inputs)
    res = run_bass_kernel_spmd(
        nc, in_maps, core_ids=list(range(NCORES)), trace=trace, **kwargs
    )
    full = np.concatenate([res.results[c]["out"] for c in range(NCORES)], axis=0)
    return full, res


def kernel(**inputs) -> np.ndarray:
    out, _ = _run(inputs, trace=False)
    return out


# revision 42
# speedup vs baseline: 1.3905x; 1.0588x over previous
"""GAT/GCN message-passing layer on 8 Trainium2 NeuronCores.

Math: the reference computes, per query node i,
    e[i,j]   = f_src[i] + f_dst[j]           (GAT additive attention, masked by Ahat>0)
    attn     = softmax_j(e masked)
    out      = relu(attn @ h_prime)
The f_src[i] term is constant along the softmax axis, so it cancels:
    attn[i,j] = Ahat[i,j]*exp(f_dst[j]) / sum_k Ahat[i,k]*exp(f_dst[k])
With g = exp(f_dst) the whole layer is one GEMM over the adjacency:
    out = relu( (Ahat @ [g*h_prime | g])[:, :256] / (Ahat @ [g*h_prime | g])[:, 256:] )
where h_prime = node_feats @ w and f_dst = node_feats @ (w @ w_a @ a[2:4]).

Sharding: 1D row partition of query nodes.  Each of the 8 cores owns 1024 rows
of Ahat (fed pre-transposed, [8192, 1024], so the contraction axis j lands on
SBUF partitions) and computes its 1024x256 slice of the output.  The small
tensors (node_feats^T, [w | u]) are replicated; each core recomputes the
B = [g*h_prime | g] panel locally, which is cheaper than a collective.

The adjacency is binary so bf16 is lossless for it; everything else is bf16 in
/ fp32-PSUM accumulate (measured end-to-end rel-err ~2e-3 vs fp32 reference).

A quirk this kernel works around everywhere: this walrus accepts only ONE sync
wait per instruction, so the dataflow is arranged so every instruction has at
most one cross-engine dependency (single-engine op chains, ACT-issued DMAs
that inherit the scalar sequencer's observed clock, and read/write "absorber"
ops ahead of DMAs that would otherwise need two waits).
"""

import os
import sys

import numpy as np

sys.path.insert(0, "/opt/trn_rl_repo")

import concourse.bass as bass  # noqa: E402
import concourse.tile as tile  # noqa: E402
from concourse import mybir  # noqa: E402
from concourse.bass_utils import run_bass_kernel_spmd  # noqa: E402
from concourse.tile import add_dep_helper  # noqa: E402

N = 8192
F = 256  # in_features == out_features
FE = F + 1  # h_prime columns + the g column
NCORES = 8
ROWS = N // NCORES  # 1024 output rows per core
P = 128
NJ = N // P  # 64 contraction blocks
NI = ROWS // P  # 8 output-row blocks per core

BF = mybir.dt.bfloat16

# j-block layout of the adjacency stream:
#   pinned tiles: j-blocks 0..JSTART-1 stay resident (PJ j-blocks per tile)
#   rotating ring: j-blocks JSTART..63 through NBUF slots (CJ j-blocks per DMA)
PJ = 8
NPIN = 4
JSTART = NPIN * PJ  # 32
CJ = 2
NROT = (NJ - JSTART) // CJ  # 16
NBUF = 8

_CACHE = {}


def _build():
    nc = bass.Bass(
        "TRN2",
        target_bir_lowering=False,
        debug=False,
        enable_asserts=True,
        num_devices=NCORES,
    )
    aT = nc.dram_tensor("aT", [N, ROWS], BF, kind="ExternalInput").ap()
    # params = [w | u | node_feats^T], all bf16, loaded in column chunks
    params = nc.dram_tensor("params", [F, FE + N], BF, kind="ExternalInput").ap()
    out = nc.dram_tensor("out", [ROWS, F], mybir.dt.float32, kind="ExternalOutput").ap()

    with tile.TileContext(nc) as tc:
        _body(tc, aT, params, out)
    return nc


def _body(tc, aT, params, out):
    nc = tc.nc
    f32 = mybir.dt.float32
    Exp = mybir.ActivationFunctionType.Exp

    with (
        tc.tile_pool(name="consts", bufs=1) as consts,
        tc.tile_pool(name="apool", bufs=1) as apool,
        tc.tile_pool(name="gpool", bufs=4) as gpool,
        tc.tile_pool(name="opool", bufs=1) as opool,
        tc.tile_pool(name="rpool", bufs=8) as rpool,
        tc.tile_pool(name="psum", bufs=1, space="PSUM") as psum,
    ):
        # ---- loads ---------------------------------------------------------
        # params in NCHP column chunks, ACT-issued so the prefix matmuls'
        # dependencies (chunk DMA + PSUM-bank WAR + B reads) all collapse onto
        # the single Activation semaphore.
        params_sb = consts.tile([P, 2, FE + N], BF, tag="params")
        NCHP = 8
        csize = (FE + N) // NCHP  # 1056 + remainder on chunk 0
        bounds = [0] + [FE + N - (NCHP - 1 - c) * csize for c in range(NCHP)]
        for c in range(NCHP):
            lo, hi = bounds[c], bounds[c + 1]
            nc.scalar.dma_start(
                params_sb[:, :, lo:hi],
                params[:, lo:hi].rearrange("(o p) f -> p o f", p=P),
            )
        wext_sb = params_sb[:, :, 0:FE]
        nfT_sb = [params_sb[:, kb, FE : FE + N] for kb in range(2)]

        # Adjacency: 4 pinned 4MB loads (j-blocks 0..31) + rotating ring.
        pinned = []
        for t in range(NPIN):
            pt = consts.tile([P, PJ * ROWS], BF, tag=f"aTp{t}", name=f"aTp{t}")
            pinned.append(pt)
            nc.sync.dma_start(
                pt.rearrange("p (o f) -> p o f", f=ROWS),
                aT[t * PJ * P : (t + 1) * PJ * P, :].rearrange(
                    "(o p) f -> p o f", p=P
                ),
            )
        rot = []
        for rc in range(NBUF):
            rt = apool.tile([P, CJ * ROWS], BF, tag=f"aTr{rc % NBUF}", name=f"aTr{rc}")
            rot.append(rt)
            nc.sync.dma_start(
                rt.rearrange("p (o f) -> p o f", f=ROWS),
                aT[(JSTART + rc * CJ) * P : (JSTART + (rc + 1) * CJ) * P, :].rearrange(
                    "(o p) f -> p o f", p=P
                ),
            )

        def a_block(j):
            """SBUF [128, 128] lhsT view of adjacency j-block, i-block i."""
            if j < JSTART:
                t = pinned[j // PJ]
                o = j % PJ
            else:
                t = rot[(j - JSTART) // CJ]
                o = (j - JSTART) % CJ
            return t, o

        # ---- PSUM accumulators --------------------------------------------
        acc = [
            psum.tile([P, FE], f32, tag=f"acc{i}", name=f"acc{i}") for i in range(NI)
        ]

        # ---- prefix: B[j] = [g*h_prime | g], all 64 j-blocks ---------------
        # h' matmuls borrow PSUM banks 0/1; those banks' accumulation groups
        # start at j=JSTART in the main stream (their earlier j-terms are
        # backfilled at the end from the pinned tiles) so the bank WAR
        # against the last B-prep doesn't stall the in-order PE queue.
        B_all = consts.tile([P, NJ * FE], BF, tag="B")
        btile = [B_all[:, j * FE : (j + 1) * FE] for j in range(NJ)]
        G = consts.tile([P, NJ], f32, tag="G")
        prev_act = None
        for j in range(NJ):
            hp = acc[j % 2]
            for kb in range(2):
                nc.tensor.matmul(
                    hp[:],
                    lhsT=nfT_sb[kb][:, j * P : (j + 1) * P],
                    rhs=wext_sb[:, kb, :],
                    start=(kb == 0),
                    stop=(kb == 1),
                )
            b = btile[j]
            gj = G[:, j : j + 1]
            ex = nc.scalar.activation(gj, hp[:, F : F + 1], Exp)
            if prev_act is not None:
                # keep B-prep in emission order on ACT; a scheduler shuffle
                # makes some of these ops pick up a second (same-sem) wait
                add_dep_helper(ex.ins, prev_act.ins, sync=False, reason="act order")
            prev_act = nc.scalar.mul(b[:, 0:F], hp[:, 0:F], gj)
            if j % 8 == 7:
                # one strided cast-copy drops this 8-group's g column into B
                c0 = j - 7
                prev_act = nc.scalar.copy(
                    B_all[:, c0 * FE + F : (j + 1) * FE : FE], G[:, c0 : j + 1]
                )

        # ---- main stream ---------------------------------------------------
        last_mm = None
        refills = []
        scr_last = None
        for j in range(NJ):
            t, o = a_block(j)
            ilist = range(NI) if j >= JSTART else range(2, NI)
            for i in ilist:
                first = j == 0 or (j == JSTART and i < 2)
                last_mm = nc.tensor.matmul(
                    acc[i][:],
                    lhsT=t[:, o * ROWS + i * P : o * ROWS + (i + 1) * P],
                    rhs=btile[j][:],
                    start=first,
                    stop=(j == NJ - 1 and i >= 2),
                )
            # ring refill bookkeeping (rotating region only)
            if j >= JSTART and (j - JSTART) % CJ == CJ - 1:
                rc = (j - JSTART) // CJ
                nxt = rc + NBUF
                if nxt < NROT:
                    at = rot[rc]
                    # single-wait absorbers: ACT read (old DMA dep) + ACT
                    # zero-write (PE readers dep); ACT-issued refill then
                    # needs only one wait.
                    head = at[:, 0:1]
                    tail = at[:, CJ * ROWS - 1 : CJ * ROWS]
                    scr = gpool.tile([P, 1], BF, tag="scr")
                    scr_last = scr
                    rd = nc.scalar.copy(scr[:], head)
                    wz = nc.scalar.mul(tail, tail, 0.0)
                    add_dep_helper(wz.ins, rd.ins, sync=False, reason="rd<wz")
                    at_new = apool.tile(
                        [P, CJ * ROWS], BF, tag=f"aTr{nxt % NBUF}", name=f"aTr{nxt}"
                    )
                    rot.append(at_new)
                    refill = nc.scalar.dma_start(
                        at_new.rearrange("p (o f) -> p o f", f=ROWS),
                        aT[
                            (JSTART + nxt * CJ) * P : (JSTART + (nxt + 1) * CJ) * P, :
                        ].rearrange("(o p) f -> p o f", p=P),
                    )
                    add_dep_helper(refill.ins, wz.ins, sync=False, reason="rf<wz")
                    refills.append(refill)

        # backfill: banks 0/1 take their j<JSTART terms from the pinned tiles
        for j in range(JSTART):
            t, o = a_block(j)
            for i in range(2):
                last_mm = nc.tensor.matmul(
                    acc[i][:],
                    lhsT=t[:, o * ROWS + i * P : o * ROWS + (i + 1) * P],
                    rhs=btile[j][:],
                    start=False,
                    stop=(j == JSTART - 1),
                )

        # ---- epilogue: out[i] = relu(acc[i][:, :F] / acc[i][:, F]) ---------
        # all DVE so each op carries at most one (PE) wait; the warm read
        # advances DVE's observed ACT clock first.
        # denominators: ACT copies each bank's g-sum into one SBUF tile (one
        # PE wait each; avoids DVE-reads-PSUM bank deps), one DVE reciprocal.
        denom8 = rpool.tile([P, NI], f32, tag="denom8")
        denom_last = None
        for i in range(NI):
            denom_last = nc.scalar.copy(denom8[:, i : i + 1], acc[i][:, F : F + 1])
        recip8 = rpool.tile([P, NI], f32, tag="recip8")
        nc.vector.reciprocal(recip8[:], denom8[:])
        # sacrificial same-proc read: soaks up the redundant DVE wait Tile
        # pins on the first consumer of recip8
        rscr = rpool.tile([P, NI], f32, tag="rscr")
        nc.vector.tensor_copy(rscr[:], recip8[:])
        otile = opool.tile([P, NI * F], f32, tag="o")
        last_dve = None
        stores = []
        for i in range(NI):
            o = otile[:, i * F : (i + 1) * F]
            nc.vector.tensor_scalar_mul(o, acc[i][:, 0:F], recip8[:, i : i + 1])
            last_dve = nc.vector.tensor_scalar_max(o, o, 0.0)
            # per-bank SWDGE store right after its relu: overlaps the tail
            stores.append(nc.gpsimd.dma_start(out[i * P : (i + 1) * P, :], o))

        # Funnel every proc's final tick into SP via single-wait nops so the
        # kernel-tail drain (which otherwise aggregates ~19 sem waits, far
        # over walrus's cap) has nothing left to wait on.
        for dep in [*refills[-NBUF:], *stores, last_mm, denom_last, last_dve]:
            nop = nc.sync.nop(nofuse=True, hint="tail_funnel")
            add_dep_helper(nop.ins, dep.ins, reason="tail funnel")


def _prep_inputs(node_feats, Ahat, w, w_a, a):
    node_feats = np.asarray(node_feats, dtype=np.float32)
    Ahat = np.asarray(Ahat, dtype=np.float32)
    w = np.asarray(w, dtype=np.float32)
    w_a = np.asarray(w_a, dtype=np.float32)
    a = np.asarray(a, dtype=np.float32)

    u = w @ (w_a @ a[2:4])  # [256, 1]
    params = np.concatenate([w, u, node_feats.T], axis=1).astype("bfloat16")
    params = np.ascontiguousarray(params)  # [256, 257 + 8192]

    in_maps = []
    for c in range(NCORES):
        aT_c = np.ascontiguousarray(
            Ahat[c * ROWS : (c + 1) * ROWS, :].T.astype("bfloat16")
        )
        in_maps.append({"aT": aT_c, "params": params})
    return in_maps


def _run(inputs, trace=False, **kwargs):
    if "nc" not in _CACHE:
        _CACHE["nc"] = _build()
    nc = _CACHE["nc"]
    in_maps = _prep_inputs(**inputs)
    res = run_bass_kernel_spmd(
        nc, in_maps, core_ids=list(range(NCORES)), trace=trace, **kwargs
    )
    full = np.concatenate([res.results[c]["out"] for c in range(NCORES)], axis=0)
    return full, res


def kernel(**inputs) -> np.ndarray:
    out, _ = _run(inputs, trace=False)
    return out
